# revision 1
# baseline (speedup 1.0000x reference)
"""CrossAttention Trainium2 kernel (8-core SPMD).

Sharding: core c = (b, g) with b = c // 2 (batch), g = c % 2 (head group of 8).
Each core computes the full attention + partial output projection for its
(batch, 8-head group); the host sums the two partial o-proj results per batch.

Per-core device pipeline (all matmuls fp32r, N=512):
  1. PE-transpose x[b], enc[b] -> xT, eT (C on partitions).
  2. Projections in natural layout: Q,K (T part, 8h x 64d free), V likewise;
     l2-norm (free-dim reduce) + partial rotary applied in natural layout.
  3. PE-transpose Q,K -> qT,kT (head-dims on partitions, T free).
  4. scoresT[k,q] = K @ Q^T accumulated in PSUM with PE-transposed bias tiles;
     exp on ACT; causal masking via memset + triangular-mask multiply;
     AV via lhsT = [V | ones] giving y^T and softmax denominators in one pass.
  5. Normalize y^T by the broadcast reciprocal denominator; o-proj from the
     head-pair-stacked y^T; DMA partial (T, C) result out.
"""

import os
import sys
from contextlib import ExitStack

import numpy as np

if not os.path.isdir(os.path.join(os.path.dirname(os.path.abspath(__file__)), "concourse")):
    for _p in ("/opt/trn_rl_repo",):
        if os.path.isdir(_p) and _p not in sys.path:
            sys.path.insert(0, _p)

import concourse.bass as bass  # noqa: E402
import concourse.tile as tile  # noqa: E402
from concourse import bacc, mybir  # noqa: E402
from concourse.bass_utils import run_bass_kernel_spmd  # noqa: E402

B, T, C = 4, 1024, 1024
H, KV, D = 16, 8, 64
L = 32
HG = 8          # heads per group (= kv heads; local head l uses kv head l)
NG = 2          # head groups
QK_NORM_SCALE = 10.0
DS = float(D) ** -0.5
SCALE_Q = DS * DS / QK_NORM_SCALE   # folded into q's rsqrt(norm) factor

F32 = mybir.dt.float32
F32R = mybir.dt.float32r

NT = T // 128   # 8 T-tiles
NC_ = C // 128  # 8 C-tiles


def r(ap):
    return ap.bitcast(F32R)


def build_program():
    nc = bacc.Bacc(
        "TRN2",
        target_bir_lowering=False,
        debug=False,
        enable_asserts=False,
        num_devices=8,
    )

    def din(name, shape):
        return nc.dram_tensor(name, shape, F32, kind="ExternalInput").ap()

    xb = din("xb", (T, C))
    eb = din("eb", (T, C))
    wq = din("wq", (C, HG * D))
    wk = din("wk", (C, KV * D))
    wv = din("wv", (C, KV * D))
    wo = din("wo", (HG * D, C))
    bias = nc.dram_tensor("bias", (HG, T, T), mybir.dt.bfloat16,
                          kind="ExternalInput").ap()
    cfq = din("cfq", (T, D))
    seq_ = din("seq", (T, L // 2))
    soq = din("soq", (T, L // 2))
    cfk = din("cfk", (T, D))
    sek = din("sek", (T, L // 2))
    sok = din("sok", (T, L // 2))
    cfv = din("cfv", (T, D))
    sev = din("sev", (T, L // 2))
    sov = din("sov", (T, L // 2))
    identf = din("identf", (128, 128))
    tri = din("tri", (128, 128))
    out_d = nc.dram_tensor("out", (T, C), F32, kind="ExternalOutput").ap()

    with tile.TileContext(nc) as tc, ExitStack() as ctx:
        const = ctx.enter_context(tc.tile_pool(name="const", bufs=1))
        persist = ctx.enter_context(tc.tile_pool(name="persist", bufs=1))

        # ---- constants ----
        identr = const.tile([128, 128], F32R, tag="identr")
        nc.sync.dma_start(identr[:], r(identf))
        identb = const.tile([128, 128], mybir.dt.bfloat16, tag="identb")
        nc.vector.tensor_copy(identb[:], identr[:].bitcast(F32))

        natp_ctx = ExitStack()
        natp_outer = natp_ctx.enter_context(tc.tile_pool(name="natp", bufs=2))
        nats = {}

        def load_nat(phase, srcd, half):
            nat = natp_outer.tile([128, 4 * C], F32R, tag="nat",
                                  name=f"nat{phase}{half}")
            nat3 = nat.rearrange("p (tt c) -> p tt c", tt=4)
            nc.sync.dma_start(
                nat3,
                r(srcd[half * 512:(half + 1) * 512, :]
                  .rearrange("(tt p) c -> p tt c", p=128)))
            nats[(phase, half)] = nat3

        load_nat("x", xb, 0)
        load_nat("x", xb, 1)

        # rope constants: (T, n) -> (128, NT, n); loaded later (DMA order)
        rope_sb = {}

        def load_rope_consts():
            for nm, ap_, w in (
                ("cfq", cfq, D), ("seq", seq_, 16), ("soq", soq, 16),
                ("cfk", cfk, D), ("sek", sek, 16), ("sok", sok, 16),
                ("cfv", cfv, D), ("sev", sev, 16), ("sov", sov, 16),
            ):
                t_ = const.tile([128, NT * w], F32, tag=nm, name=nm)
                t3 = t_.rearrange("p (tt d) -> p tt d", tt=NT)
                nc.sync.dma_start(t3, ap_.rearrange("(tt p) d -> p tt d", p=128))
                rope_sb[nm] = t3

        # persistent across attention: wo (loaded later), qT/kT, va
        wo_t = persist.tile([128, 4 * C], F32R, tag="wo", name="wo_t")
        wo_sb = wo_t.rearrange("p (pl c) -> p pl c", pl=4)

        def load_wo_trim():
            nc.sync.dma_start(wo_sb, r(wo.rearrange("(pl p) c -> p pl c", p=128)))
        qT = {(pl, h): persist.tile([128, 512], F32R, tag=f"qT{pl}_{h}",
                                    name=f"qT{pl}_{h}")
              for pl in range(4) for h in range(2)}
        kT = {(pl, h): persist.tile([128, 512], F32R, tag=f"kT{pl}_{h}",
                                    name=f"kT{pl}_{h}")
              for pl in range(4) for h in range(2)}
        va = [persist.tile([128, HG * 65], F32R, tag=f"va{tt}", name=f"va{tt}") for tt in range(NT)]

        def rope_inplace(v3, tt, cf, se, so, smallp):
            """v3: (128, HG, d) SBUF view; applies partial rotary in place."""
            ev = v3[:, :, 0:L:2]
            od = v3[:, :, 1:L:2]
            se_b = rope_sb[se][:, tt].unsqueeze(1).broadcast_to([128, HG, 16])
            so_b = rope_sb[so][:, tt].unsqueeze(1).broadcast_to([128, HG, 16])
            cf_b = rope_sb[cf][:, tt].unsqueeze(1).broadcast_to([128, HG, D])
            tmp_e = smallp.tile([128, HG * 16], F32, tag="tmpe", name="tmpe")
            tmp_o = smallp.tile([128, HG * 16], F32, tag="tmpo", name="tmpo")
            te3 = tmp_e.rearrange("p (h d) -> p h d", h=HG)
            to3 = tmp_o.rearrange("p (h d) -> p h d", h=HG)
            nc.vector.tensor_mul(te3, od, se_b)
            nc.vector.tensor_mul(to3, ev, so_b)
            nc.gpsimd.tensor_mul(v3[:, :, 0:D], v3[:, :, 0:D], cf_b)
            nc.vector.tensor_sub(ev, ev, te3)
            nc.vector.tensor_add(od, od, to3)

        def flush_qn(qns, ttg, tpsum, dstT):
            """PE-transpose 4 ready qn tiles into dstT[pl][:, ttg*512:]."""
            for pl in range(4):
                ps4 = tpsum.tile([128, 512], F32, tag="tps", name="tps")
                for tti in range(4):
                    nc.tensor.matmul(
                        r(ps4[:, tti * 128:(tti + 1) * 128]),
                        qns[tti][:, pl * 128:(pl + 1) * 128],
                        identr[:], is_transpose=True, start=True, stop=True,
                    )
                nc.any.tensor_copy(dstT[(pl, ttg)][:], ps4[:])

        def norm_rope_transpose(ps, tt, which, smallp, sqp, rotp, tpsum, dstT):
            """ps: (128 T, 512) psum of raw projections. Normalizes per head,
            applies rope; returns the qn tile."""
            sq = sqp.tile([128, HG * D], F32, tag="sq", name="sq")
            nc.scalar.square(sq[:], ps[:])
            ss = smallp.tile([128, HG], F32, tag="ss", name="ss")
            nc.vector.tensor_reduce(
                ss[:], sq.rearrange("p (h d) -> p h d", h=HG),
                axis=mybir.AxisListType.X, op=mybir.AluOpType.add,
            )
            inv = smallp.tile([128, HG], F32, tag="inv", name="inv")
            nc.vector.reciprocal(inv[:], ss[:])
            rs = smallp.tile([128, HG], F32, tag="rs", name="rs")
            scl = SCALE_Q * SCALE_Q if which == "q" else 1.0
            nc.scalar.activation(
                rs[:], inv[:], mybir.ActivationFunctionType.Sqrt,
                bias=0.0, scale=scl,
            )
            qn = rotp.tile([128, HG * D], F32R, tag="qn", name="qn")
            d3 = qn.rearrange("p (h d) -> p h d", h=HG)
            nc.vector.tensor_mul(
                d3, ps.rearrange("p (h d) -> p h d", h=HG),
                rs[:].unsqueeze(2).broadcast_to([128, HG, D]),
            )
            if which == "q":
                rope_inplace(d3, tt, "cfq", "seq", "soq", smallp)
            else:
                rope_inplace(d3, tt, "cfk", "sek", "sok", smallp)
            return qn

        # ---- x phase: transpose x -> xT, project Q, -> qT ----
        for phase in ("x", "e"):
            with tc.tile_pool(name="srcT", bufs=1) as srcTp, \
                 tc.tile_pool(name="wp", bufs=1) as wp, \
                 tc.tile_pool(name="projp", bufs=4, space="PSUM") as projp, \
                 tc.tile_pool(name="tpsum", bufs=3, space="PSUM") as tpsum, \
                 tc.tile_pool(name="smallp", bufs=6) as smallp, \
                 tc.tile_pool(name="sqp", bufs=2) as sqp, \
                 tc.tile_pool(name="rotp", bufs=5) as rotp:
                srcT = [srcTp.tile([128, T], F32R, tag=f"sT{cb}", name=f"sT{cb}")
                        for cb in range(NC_)]
                for ttg in range(2):
                    nat3 = nats[(phase, ttg)]
                    for cb in range(NC_):
                        ps4 = tpsum.tile([128, 512], F32, tag="tps",
                                         name="tps")
                        for tti in range(4):
                            nc.tensor.matmul(
                                r(ps4[:, tti * 128:(tti + 1) * 128]),
                                nat3[:, tti, cb * 128:(cb + 1) * 128],
                                identr[:], is_transpose=True,
                                start=True, stop=True,
                            )
                        nc.any.tensor_copy(
                            srcT[cb][:, ttg * 512:(ttg + 1) * 512], ps4[:]
                        )
                if phase == "x":
                    wq_t = wp.tile([128, NC_ * 512], F32R, tag="wq", name="wq_t")
                    wq_sb = wq_t.rearrange("p (cb n) -> p cb n", cb=NC_)
                    nc.sync.dma_start(
                        wq_sb, r(wq.rearrange("(cb p) n -> p cb n", p=128)))
                    load_rope_consts()
                    load_nat("e", eb, 0)
                    load_nat("e", eb, 1)
                    load_wo_trim()
                    qns = []
                    for tt in range(NT):
                        ps = projp.tile([128, 512], F32, tag="proj", name="proj")
                        for cb in range(NC_):
                            nc.tensor.matmul(
                                ps[:], r(srcT[cb][:, tt * 128:(tt + 1) * 128]),
                                r(wq_sb[:, cb]),
                                start=(cb == 0), stop=(cb == NC_ - 1),
                            )
                        qns.append(norm_rope_transpose(ps, tt, "q", smallp,
                                                       sqp, rotp, tpsum, qT))
                        if tt % 4 == 3:
                            flush_qn(qns[-4:], tt // 4, tpsum, qT)
                else:
                    wk_t = wp.tile([128, NC_ * 512], F32R, tag="wk", name="wk_t")
                    wk_sb = wk_t.rearrange("p (cb n) -> p cb n", cb=NC_)
                    nc.sync.dma_start(
                        wk_sb, r(wk.rearrange("(cb p) n -> p cb n", p=128)))
                    wv_t = wp.tile([128, NC_ * 512], F32R, tag="wv", name="wv_t")
                    wv_sb = wv_t.rearrange("p (cb n) -> p cb n", cb=NC_)
                    nc.sync.dma_start(
                        wv_sb, r(wv.rearrange("(cb p) n -> p cb n", p=128)))
                    kns = []
                    for tt in range(NT):
                        ps = projp.tile([128, 512], F32, tag="proj", name="proj")
                        for cb in range(NC_):
                            nc.tensor.matmul(
                                ps[:], r(srcT[cb][:, tt * 128:(tt + 1) * 128]),
                                r(wk_sb[:, cb]),
                                start=(cb == 0), stop=(cb == NC_ - 1),
                            )
                        kns.append(norm_rope_transpose(ps, tt, "k", smallp,
                                                       sqp, rotp, tpsum, kT))
                        if tt % 4 == 3:
                            flush_qn(kns[-4:], tt // 4, tpsum, kT)
                        # V: no norm; pack into 65-stride with ones column
                        psv = projp.tile([128, 512], F32, tag="proj", name="projv")
                        for cb in range(NC_):
                            nc.tensor.matmul(
                                psv[:], r(srcT[cb][:, tt * 128:(tt + 1) * 128]),
                                r(wv_sb[:, cb]),
                                start=(cb == 0), stop=(cb == NC_ - 1),
                            )
                        v3 = va[tt].rearrange("p (h e) -> p h e", h=HG)
                        nc.vector.tensor_copy(
                            v3[:, :, 0:D],
                            psv.rearrange("p (h d) -> p h d", h=HG),
                        )
                        nc.vector.memset(v3[:, :, D:D + 1].bitcast(F32), 1.0)
                        rope_inplace(v3, tt, "cfv", "sev", "sov", smallp)

        natp_ctx.close()

        # ---- attention (qg-outer) + interleaved o-proj ----
        ys = {}
        for pl in range(4):
            for qg in range(2):
                ys[(pl, qg)] = persist.tile([128, 512], F32R,
                                            tag=f"ys{pl}_{qg}",
                                            name=f"ys{pl}_{qg}")

        with tc.tile_pool(name="biasp", bufs=2) as biasp, \
             tc.tile_pool(name="attp", bufs=6) as attp, \
             tc.tile_pool(name="spsum", bufs=4, space="PSUM") as spsum, \
             tc.tile_pool(name="ypsum", bufs=2, space="PSUM") as ypsum, \
             tc.tile_pool(name="opsum", bufs=2, space="PSUM") as opsum, \
             tc.tile_pool(name="outp", bufs=2) as outp, \
             tc.tile_pool(name="smalle", bufs=4) as smalle:

            def oproj(tt):
                ot = outp.tile([128, C], F32, tag="ot", name="ot")
                qg = tt // 4
                for cg in range(2):
                    pso = opsum.tile([128, 512], F32, tag="pso", name="pso")
                    for pl in range(4):
                        nc.tensor.matmul(
                            pso[:],
                            r(ys[(pl, qg)][:, (tt % 4) * 128:(tt % 4 + 1) * 128]),
                            r(wo_sb[:, pl, cg * 512:(cg + 1) * 512]),
                            start=(pl == 0), stop=(pl == 3),
                        )
                    nc.vector.tensor_copy(ot[:, cg * 512:(cg + 1) * 512], pso[:])
                nc.sync.dma_start(out_d[tt * 128:(tt + 1) * 128, :], ot[:])

            for qg in range(2):
                q0 = qg * 512
                nkt = qg * 4 + 4
                qts = range(qg * 4, qg * 4 + 4)
                for lb in range(0, HG, 2):      # head blocks of 2
                    bt = biasp.tile([128, nkt * 2 * 512], mybir.dt.bfloat16,
                                    tag=f"bias{qg}", name=f"bias{qg}_{lb}")
                    bt4 = bt.rearrange("p (h kt q) -> p h kt q", kt=nkt, h=2)
                    for h_ in range(2):
                        nc.sync.dma_start(
                            bt4[:, h_],
                            bias[lb + h_, 0:nkt * 128, q0:q0 + 512]
                            .rearrange("(kt p) q -> p kt q", p=128),
                        )
                    for l4 in range(2):
                        l = lb + l4
                        pl, sub = l // 2, l % 2
                        po = 64 * sub
                        psy = ypsum.tile([65, 512], F32, tag="psy", name="psy")
                        for kt in range(nkt):
                            pss = spsum.tile([128, 512], F32, tag="pss",
                                             name="pss")
                            nc.tensor.matmul(
                                pss[:],
                                r(kT[(pl, kt // 4)][po:po + 64,
                                                    (kt % 4) * 128:(kt % 4 + 1) * 128]),
                                r(qT[(pl, qg)][po:po + 64, :]),
                                start=True, stop=False,
                            )
                            nc.tensor.matmul(
                                pss[:], identb[:], bt4[:, l4, kt, :],
                                start=False, stop=True,
                            )
                            att = attp.tile([128, 512], F32R, tag="att",
                                            name="att")
                            nc.scalar.activation(
                                att[:], pss[:],
                                mybir.ActivationFunctionType.Exp,
                            )
                            nc.tensor.matmul(
                                psy[:],
                                r(va[kt][:, l * 65:(l + 1) * 65]),
                                att[:],
                                start=(kt == 0), stop=(kt == nkt - 1),
                            )
                        rcp = smalle.tile([1, 512], F32, tag="rcp", name="rcp")
                        nc.vector.reciprocal(rcp[:], psy[64:65, :])
                        rb = smalle.tile([64, 512], F32, tag="rb", name="rb")
                        nc.gpsimd.partition_broadcast(rb[:], rcp[:])
                        nc.vector.tensor_mul(
                            ys[(pl, qg)][po:po + 64, :],
                            psy[0:64, :], rb[:],
                        )
                # after all heads of this qg: o-proj for its 4 Tq tiles
                for tt in qts:
                    oproj(tt)

    nc.compile()
    return nc


def host_prep(freqs, q_scale, k_scale):
    """Build rope constant tensors (shared across cores)."""
    c = np.cos(freqs[:, 0::2]).astype(np.float32)   # (T, 16)
    s = np.sin(freqs[:, 0::2]).astype(np.float32)
    consts = {}
    for nm, scale in (("q", q_scale), ("k", k_scale), ("v", np.ones(D, np.float32))):
        scale = np.asarray(scale, np.float32)
        cf = np.empty((T, D), np.float32)
        cf[:, 0:L:2] = c * scale[0:L:2][None, :]
        cf[:, 1:L:2] = c * scale[1:L:2][None, :]
        cf[:, L:] = scale[L:][None, :]
        se = (s * scale[1:L:2][None, :]).astype(np.float32)   # mult q_odd -> even
        so = (s * scale[0:L:2][None, :]).astype(np.float32)   # mult q_even -> odd
        consts[f"cf{nm}"] = np.ascontiguousarray(cf)
        consts[f"se{nm}"] = np.ascontiguousarray(se)
        consts[f"so{nm}"] = np.ascontiguousarray(so)
    consts["identf"] = np.eye(128, dtype=np.float32)
    ii = np.arange(128)
    consts["tri"] = (ii[:, None] <= ii[None, :]).astype(np.float32)
    return consts


_NC_CACHE = {}


def get_nc():
    if "nc" not in _NC_CACHE:
        _NC_CACHE["nc"] = build_program()
    return _NC_CACHE["nc"]


def make_in_maps(x, encoded_data, freqs, attn_bias, Wq, Wk, Wv, Wo,
                 q_scale, k_scale):
    consts = host_prep(np.asarray(freqs, np.float32),
                       np.asarray(q_scale, np.float32),
                       np.asarray(k_scale, np.float32))
    import ml_dtypes
    x = np.asarray(x, np.float32)
    e = np.asarray(encoded_data, np.float32)
    ab = np.asarray(attn_bias, np.float32)
    ii = np.arange(T)
    causal = ii[None, :, None] < ii[None, None, :]   # (1, q, k): k > q masked
    abm = np.where(causal, np.float32(-30.0), ab)    # (H, q, k)
    abT = np.ascontiguousarray(abm.transpose(0, 2, 1)).astype(ml_dtypes.bfloat16)
    Wq = np.asarray(Wq, np.float32)
    Wk = np.ascontiguousarray(np.asarray(Wk, np.float32))
    Wv = np.ascontiguousarray(np.asarray(Wv, np.float32))
    Wo = np.asarray(Wo, np.float32)
    in_maps = []
    for core in range(8):
        b, g = core // 2, core % 2
        m = dict(consts)
        m["xb"] = np.ascontiguousarray(x[b])
        m["eb"] = np.ascontiguousarray(e[b])
        m["wq"] = np.ascontiguousarray(Wq[:, g * 512:(g + 1) * 512])
        m["wk"] = Wk
        m["wv"] = Wv
        m["wo"] = np.ascontiguousarray(Wo[g * 512:(g + 1) * 512, :])
        m["bias"] = np.ascontiguousarray(abT[g * HG:(g + 1) * HG])
        in_maps.append(m)
    return in_maps


def kernel(x, encoded_data, freqs, attn_bias, Wq, Wk, Wv, Wo,
           q_scale, k_scale):
    nc = get_nc()
    in_maps = make_in_maps(x, encoded_data, freqs, attn_bias,
                           Wq, Wk, Wv, Wo, q_scale, k_scale)
    res = run_bass_kernel_spmd(nc, in_maps, core_ids=list(range(8)))
    out = np.empty((B, T, C), np.float32)
    for b in range(B):
        out[b] = res.results[2 * b]["out"] + res.results[2 * b + 1]["out"]
    return out



# revision 6
# speedup vs baseline: 20.9720x; 20.9720x over previous
"""CrossAttention Trainium2 kernel (8-core SPMD).

Sharding: core c = (b, g) with b = c // 2 (batch), g = c % 2 (head group of 8).
Each core computes the full attention + partial output projection for its
(batch, 8-head group); the host sums the two partial o-proj results per batch.

Key design points (v2):
  - Host pre-transposes x/e to (C, T) and ships all matmul operands in bf16:
    projections consume xT/eT directly (no on-device PE transposes of x/e).
  - Rope dims are de-interleaved by a per-head permutation of the weight
    columns (scores are invariant under a shared q/k permutation; it is
    undone in Wo's rows for the v path), so all rope elementwise ops are
    contiguous and DVE-fast.
  - Host ships the causal-masked bias (-30 in masked positions, bf16),
    column-trimmed to the unmasked triangle and packed; fully-masked
    128-column blocks are skipped outright. Bias is added into the score
    PSUM via an identity matmul on PE (cheaper than a DVE multiply).
  - l2-norm + partial rotary in natural layout, then PE-transpose q/k (bf16);
    AV via lhsT = [V | ones] giving y^T and softmax denominators in one pass.
"""

import os
import sys
from contextlib import ExitStack

import numpy as np

if not os.path.isdir(os.path.join(os.path.dirname(os.path.abspath(__file__)), "concourse")):
    for _p in ("/opt/trn_rl_repo",):
        if os.path.isdir(_p) and _p not in sys.path:
            sys.path.insert(0, _p)

import concourse.bass as bass  # noqa: E402
import concourse.tile as tile  # noqa: E402
from concourse import bacc, mybir  # noqa: E402
from concourse.bass_utils import run_bass_kernel_spmd  # noqa: E402

B, T, C = 4, 1024, 1024
H, KV, D = 16, 8, 64
L = 32
HG = 8          # heads per group (= kv heads; local head l uses kv head l)
NG = 2          # head groups
QK_NORM_SCALE = 10.0
DS = float(D) ** -0.5
SCALE_Q = DS * DS / QK_NORM_SCALE   # folded into q's rsqrt(norm) factor

F32 = mybir.dt.float32
BF16 = mybir.dt.bfloat16

NT = T // 128   # 8 T-tiles
NC_ = C // 128  # 8 C-tiles

# packed, causal-trimmed exp(bias) widths: for query group qg, key tile kt,
# the needed query columns start at max(kt - qg*4, 0) * 128
def _col_layout(qg):
    offs, widths, qoffs = [], [], []
    o = 0
    for kt in range(qg * 4 + 4):
        i = kt - qg * 4
        w = 512 - max(i, 0) * 128
        offs.append(o)
        widths.append(w)
        qoffs.append(512 - w)
        o += w
    return offs, widths, qoffs, o

OFFS0, WID0, QOFF0, W0 = _col_layout(0)   # W0 = 1280
OFFS1, WID1, QOFF1, W1 = _col_layout(1)   # W1 = 3328


def build_program():
    nc = bacc.Bacc(
        "TRN2",
        target_bir_lowering=False,
        debug=False,
        enable_asserts=False,
        num_devices=8,
    )

    def din(name, shape, dt=BF16):
        return nc.dram_tensor(name, shape, dt, kind="ExternalInput").ap()

    xT = din("xT", (C, T))
    eT = din("eT", (C, T))
    wq = din("wq", (C, HG * D))
    wk = din("wk", (C, KV * D))
    wv = din("wv", (C, KV * D))
    wo = din("wo", (HG * D, C))
    ebias0 = din("ebias0", (HG, 128, W0))
    ebias1 = din("ebias1", (HG, 128, W1))
    ropec = din("ropec", (T, 288))
    identb_d = din("identb", (128, 128))
    out_d = nc.dram_tensor("out", (T, C), BF16, kind="ExternalOutput").ap()

    with tile.TileContext(nc) as tc, ExitStack() as ctx:
        const = ctx.enter_context(tc.tile_pool(name="const", bufs=1))
        persist = ctx.enter_context(tc.tile_pool(name="persist", bufs=1))

        # ---- constants / weights (DMA order = priority order) ----
        identb = const.tile([128, 128], BF16, tag="identb")
        nc.sync.dma_start(identb[:], identb_d)

        xT_t = persist.tile([128, NC_ * T], BF16, tag="xT")
        xT_sb = xT_t.rearrange("p (cb t) -> p cb t", cb=NC_)
        nc.sync.dma_start(xT_sb, xT.rearrange("(cb p) t -> p cb t", p=128))

        wq_t = persist.tile([128, NC_ * 512], BF16, tag="wq")
        wq_sb = wq_t.rearrange("p (cb n) -> p cb n", cb=NC_)
        nc.sync.dma_start(wq_sb, wq.rearrange("(cb p) n -> p cb n", p=128))

        ropec_t = const.tile([128, NT * 288], BF16, tag="ropec")
        ropec_sb = ropec_t.rearrange("p (tt d) -> p tt d", tt=NT)
        nc.sync.dma_start(ropec_sb, ropec.rearrange("(tt p) d -> p tt d", p=128))

        # rope constant views: [cf(64) se(16) so(16)] x (q, k, v)
        def rviews(base):
            return (ropec_sb[:, :, base:base + 64],
                    ropec_sb[:, :, base + 64:base + 80],
                    ropec_sb[:, :, base + 80:base + 96])

        rope_q, rope_k, rope_v = rviews(0), rviews(96), rviews(192)

        # persistent: qT/kT (d on partitions, t free), va (t part, h x 65)
        qT = {(pl, hf): persist.tile([128, 512], BF16, tag=f"qT{pl}_{hf}",
                                     name=f"qT{pl}_{hf}")
              for pl in range(4) for hf in range(2)}
        kT = {(pl, hf): persist.tile([128, 512], BF16, tag=f"kT{pl}_{hf}",
                                     name=f"kT{pl}_{hf}")
              for pl in range(4) for hf in range(2)}
        va = [persist.tile([128, HG * 65], BF16, tag=f"va{tt}", name=f"va{tt}")
              for tt in range(NT)]

        # exp(bias) tiles, software-prefetched
        biasp = ctx.enter_context(tc.tile_pool(name="biasp", bufs=2))
        bias_tiles = {}

        def issue_bias(qg, lb):
            if (qg, lb) in bias_tiles:
                return
            W = W0 if qg == 0 else W1
            src = ebias0 if qg == 0 else ebias1
            bt = biasp.tile([128, 2 * W], BF16, tag=f"bias{qg}",
                            name=f"bias{qg}_{lb}")
            bt3 = bt.rearrange("p (h w) -> p h w", h=2)
            nc.sync.dma_start(bt3, src[lb:lb + 2].rearrange("h p w -> p h w"))
            bias_tiles[(qg, lb)] = bt3

        def rope_inplace(v3, tt, rope_views, smallp):
            """v3: (128, HG, 64) bf16 SBUF view, rope dims de-interleaved:
            new[0:16] = old even, new[16:32] = old odd, new[32:64] untouched
            by rotation (but cf applies the per-dim scale everywhere)."""
            cf, se, so = rope_views
            ev = v3[:, :, 0:16]
            od = v3[:, :, 16:32]
            se_b = se[:, tt].unsqueeze(1).broadcast_to([128, HG, 16])
            so_b = so[:, tt].unsqueeze(1).broadcast_to([128, HG, 16])
            cf_b = cf[:, tt].unsqueeze(1).broadcast_to([128, HG, D])
            tmp_e = smallp.tile([128, HG * 16], BF16, tag="tmpe", name="tmpe")
            tmp_o = smallp.tile([128, HG * 16], BF16, tag="tmpo", name="tmpo")
            te3 = tmp_e.rearrange("p (h d) -> p h d", h=HG)
            to3 = tmp_o.rearrange("p (h d) -> p h d", h=HG)
            nc.vector.tensor_mul(te3, od, se_b)
            nc.vector.tensor_mul(to3, ev, so_b)
            nc.vector.tensor_mul(v3[:, :, 0:D], v3[:, :, 0:D], cf_b)
            nc.vector.tensor_sub(ev, ev, te3)
            nc.vector.tensor_add(od, od, to3)

        def flush_qn(qns, ttg, tpsum, dstT):
            """PE-transpose 4 ready (128t, 512d) bf16 tiles into
            dstT[(pl, ttg)] (128d, 512t)."""
            for pl in range(4):
                ps4 = tpsum.tile([128, 512], BF16, tag="tps", name="tps")
                for tti in range(4):
                    nc.tensor.matmul(
                        ps4[:, tti * 128:(tti + 1) * 128],
                        qns[tti][:, pl * 128:(pl + 1) * 128],
                        identb[:], is_transpose=True, start=True, stop=True,
                    )
                nc.scalar.copy(dstT[(pl, ttg)][:], ps4[:])

        def norm_rope(ps, tt, which, smallp, sqp, rotp):
            """ps: (128 t, 512) psum of raw q/k projections. Per-head l2
            normalization (scaled), then rope; returns bf16 tile."""
            ps3 = ps.rearrange("p (h d) -> p h d", h=HG)
            sq = sqp.tile([128, HG * D], F32, tag="sq", name="sq")
            sq3 = sq.rearrange("p (h d) -> p h d", h=HG)
            nc.scalar.square(sq[:], ps[:])
            ss = smallp.tile([128, HG], F32, tag="ss", name="ss")
            nc.vector.tensor_reduce(
                ss[:], sq3, axis=mybir.AxisListType.X, op=mybir.AluOpType.add,
            )
            inv = smallp.tile([128, HG], F32, tag="inv", name="inv")
            nc.vector.reciprocal(inv[:], ss[:])
            rs = smallp.tile([128, HG], F32, tag="rs", name="rs")
            scl = SCALE_Q * SCALE_Q if which == "q" else 1.0
            nc.scalar.activation(
                rs[:], inv[:], mybir.ActivationFunctionType.Sqrt,
                bias=0.0, scale=scl,
            )
            qn = rotp.tile([128, HG * D], BF16, tag="qn", name="qn")
            d3 = qn.rearrange("p (h d) -> p h d", h=HG)
            nc.vector.tensor_mul(
                d3, ps3, rs[:].unsqueeze(2).broadcast_to([128, HG, D]),
            )
            rope_inplace(d3, tt, rope_q if which == "q" else rope_k, smallp)
            return qn

        # ---- projection phases ----
        with tc.tile_pool(name="projp", bufs=4, space="PSUM") as projp, \
             tc.tile_pool(name="tpsum", bufs=3, space="PSUM") as tpsum, \
             tc.tile_pool(name="smallp", bufs=6) as smallp, \
             tc.tile_pool(name="sqp", bufs=2) as sqp, \
             tc.tile_pool(name="rotp", bufs=5) as rotp:

            # phase X: Q = xT.T @ wq, per 128-row t-tile
            qns = []
            for tt in range(NT):
                ps = projp.tile([128, 512], F32, tag="proj", name="proj")
                for cb in range(NC_):
                    nc.tensor.matmul(
                        ps[:], xT_sb[:, cb, tt * 128:(tt + 1) * 128],
                        wq_sb[:, cb],
                        start=(cb == 0), stop=(cb == NC_ - 1),
                    )
                qns.append(norm_rope(ps, tt, "q", smallp, sqp, rotp))
                if tt % 4 == 3:
                    flush_qn(qns[-4:], tt // 4, tpsum, qT)
                if tt == 0:
                    # queue remaining input DMAs behind the x/wq loads
                    eT_t = persist.tile([128, NC_ * T], BF16, tag="eT")
                    eT_sb = eT_t.rearrange("p (cb t) -> p cb t", cb=NC_)
                    nc.sync.dma_start(
                        eT_sb, eT.rearrange("(cb p) t -> p cb t", p=128))
                    wk_t = persist.tile([128, NC_ * 512], BF16, tag="wk")
                    wk_sb = wk_t.rearrange("p (cb n) -> p cb n", cb=NC_)
                    nc.sync.dma_start(
                        wk_sb, wk.rearrange("(cb p) n -> p cb n", p=128))
                    wv_t = persist.tile([128, NC_ * 512], BF16, tag="wv")
                    wv_sb = wv_t.rearrange("p (cb n) -> p cb n", cb=NC_)
                    nc.sync.dma_start(
                        wv_sb, wv.rearrange("(cb p) n -> p cb n", p=128))
                    wo_t = persist.tile([128, 4 * C], BF16, tag="wo")
                    wo_sb = wo_t.rearrange("p (pl c) -> p pl c", pl=4)
                    nc.sync.dma_start(
                        wo_sb, wo.rearrange("(pl p) c -> p pl c", p=128))
                    issue_bias(0, 0)
                    issue_bias(0, 2)

            # phase E: K and V from eT
            kns = []
            for tt in range(NT):
                ps = projp.tile([128, 512], F32, tag="proj", name="proj")
                for cb in range(NC_):
                    nc.tensor.matmul(
                        ps[:], eT_sb[:, cb, tt * 128:(tt + 1) * 128],
                        wk_sb[:, cb],
                        start=(cb == 0), stop=(cb == NC_ - 1),
                    )
                kns.append(norm_rope(ps, tt, "k", smallp, sqp, rotp))
                if tt % 4 == 3:
                    flush_qn(kns[-4:], tt // 4, tpsum, kT)
                psv = projp.tile([128, 512], F32, tag="proj", name="projv")
                for cb in range(NC_):
                    nc.tensor.matmul(
                        psv[:], eT_sb[:, cb, tt * 128:(tt + 1) * 128],
                        wv_sb[:, cb],
                        start=(cb == 0), stop=(cb == NC_ - 1),
                    )
                v3 = va[tt].rearrange("p (h e) -> p h e", h=HG)
                nc.vector.tensor_copy(
                    v3[:, :, 0:D], psv.rearrange("p (h d) -> p h d", h=HG))
                nc.vector.memset(v3[:, :, D:D + 1], 1.0)
                rope_inplace(v3, tt, rope_v, smallp)

        # ---- attention (qg-outer) + interleaved o-proj ----
        ys = {}
        for pl in range(4):
            for qg in range(2):
                ys[(pl, qg)] = persist.tile([128, 512], BF16,
                                            tag=f"ys{pl}_{qg}",
                                            name=f"ys{pl}_{qg}")

        with tc.tile_pool(name="attp", bufs=6) as attp, \
             tc.tile_pool(name="spsum", bufs=4, space="PSUM") as spsum, \
             tc.tile_pool(name="ypsum", bufs=2, space="PSUM") as ypsum, \
             tc.tile_pool(name="opsum", bufs=2, space="PSUM") as opsum, \
             tc.tile_pool(name="outp", bufs=2) as outp, \
             tc.tile_pool(name="smalle", bufs=4) as smalle:

            def oproj(tt):
                ot = outp.tile([128, C], BF16, tag="ot", name="ot")
                qg = tt // 4
                for cg in range(2):
                    pso = opsum.tile([128, 512], F32, tag="pso", name="pso")
                    for pl in range(4):
                        nc.tensor.matmul(
                            pso[:],
                            ys[(pl, qg)][:, (tt % 4) * 128:(tt % 4 + 1) * 128],
                            wo_sb[:, pl, cg * 512:(cg + 1) * 512],
                            start=(pl == 0), stop=(pl == 3),
                        )
                    nc.vector.tensor_copy(ot[:, cg * 512:(cg + 1) * 512], pso[:])
                nc.sync.dma_start(out_d[tt * 128:(tt + 1) * 128, :], ot[:])

            for qg in range(2):
                nkt = qg * 4 + 4
                offs = OFFS0 if qg == 0 else OFFS1
                wids = WID0 if qg == 0 else WID1
                qoffs = QOFF0 if qg == 0 else QOFF1
                for lb in range(0, HG, 2):      # head blocks of 2
                    issue_bias(qg, lb)
                    # prefetch the block after next (bufs=2 per qg tag)
                    nlb = lb + 2
                    if nlb < HG:
                        issue_bias(qg, nlb)
                    elif qg == 0:
                        issue_bias(1, 0)
                    bt3 = bias_tiles[(qg, lb)]
                    for l4 in range(2):
                        l = lb + l4
                        pl, sub = l // 2, l % 2
                        po = 64 * sub
                        psy = ypsum.tile([65, 512], F32, tag="psy", name="psy")
                        for kt in range(nkt):
                            w, qo, off = wids[kt], qoffs[kt], offs[kt]
                            pss = spsum.tile([128, 512], F32, tag="pss",
                                             name="pss")
                            nc.tensor.matmul(
                                pss[:, 0:w],
                                kT[(pl, kt // 4)][po:po + 64,
                                                  (kt % 4) * 128:(kt % 4 + 1) * 128],
                                qT[(pl, qg)][po:po + 64, qo:512],
                                start=True, stop=False,
                            )
                            nc.tensor.matmul(
                                pss[:, 0:w], identb[:],
                                bt3[:, l4, off:off + w],
                                start=False, stop=True,
                            )
                            att = attp.tile([128, 512], BF16, tag="att",
                                            name="att")
                            nc.scalar.activation(
                                att[:, 0:w], pss[:, 0:w],
                                mybir.ActivationFunctionType.Exp,
                            )
                            nc.tensor.matmul(
                                psy[:, qo:512],
                                va[kt][:, l * 65:(l + 1) * 65],
                                att[:, 0:w],
                                start=(kt == 0), stop=(kt == nkt - 1),
                            )
                        rcp = smalle.tile([1, 512], F32, tag="rcp", name="rcp")
                        nc.vector.reciprocal(rcp[:], psy[64:65, :])
                        rb = smalle.tile([64, 512], F32, tag="rb", name="rb")
                        nc.gpsimd.partition_broadcast(rb[:], rcp[:])
                        nc.vector.tensor_mul(
                            ys[(pl, qg)][po:po + 64, :],
                            psy[0:64, :], rb[:],
                        )
                # after all heads of this qg: o-proj for its 4 Tq tiles
                for tt in range(qg * 4, qg * 4 + 4):
                    oproj(tt)

    nc.compile()
    return nc


PERM = np.concatenate([np.arange(0, L, 2), np.arange(1, L, 2),
                       np.arange(L, D)])   # de-interleave rope pairs


def host_prep(freqs, q_scale, k_scale):
    """Rope constant tensors in the permuted dim order, packed (T, 288)."""
    c = np.cos(freqs[:, 0::2]).astype(np.float32)   # (T, 16)
    s = np.sin(freqs[:, 0::2]).astype(np.float32)
    import ml_dtypes
    ropec = np.empty((T, 288), np.float32)
    for j, scale in enumerate((q_scale, k_scale, np.ones(D, np.float32))):
        scale = np.asarray(scale, np.float32)
        cf = np.empty((T, D), np.float32)
        cf[:, 0:L:2] = c * scale[0:L:2][None, :]
        cf[:, 1:L:2] = c * scale[1:L:2][None, :]
        cf[:, L:] = scale[L:][None, :]
        se = s * scale[1:L:2][None, :]    # multiplies odd input -> even out
        so = s * scale[0:L:2][None, :]    # multiplies even input -> odd out
        base = j * 96
        ropec[:, base:base + 64] = cf[:, PERM]
        ropec[:, base + 64:base + 80] = se
        ropec[:, base + 80:base + 96] = so
    consts = {
        "ropec": ropec.astype(ml_dtypes.bfloat16),
        "identb": np.eye(128, dtype=ml_dtypes.bfloat16),
    }
    return consts


def pack_ebias(eb_g):
    """eb_g: (HG, Tk, Tq) float32 exp(bias) with causal zeros, for one head
    group. Returns packed (HG, 128, W0), (HG, 128, W1) bf16."""
    import ml_dtypes
    p0 = np.empty((HG, 128, W0), np.float32)
    p1 = np.empty((HG, 128, W1), np.float32)
    for qg, (p, offs, wids, qoffs) in enumerate(
            ((p0, OFFS0, WID0, QOFF0), (p1, OFFS1, WID1, QOFF1))):
        q0 = qg * 512
        for kt in range(qg * 4 + 4):
            o, w, qo = offs[kt], wids[kt], qoffs[kt]
            p[:, :, o:o + w] = eb_g[:, kt * 128:(kt + 1) * 128,
                                    q0 + qo:q0 + 512]
    return p0.astype(ml_dtypes.bfloat16), p1.astype(ml_dtypes.bfloat16)


_NC_CACHE = {}


def get_nc():
    if "nc" not in _NC_CACHE:
        _NC_CACHE["nc"] = build_program()
    return _NC_CACHE["nc"]


def make_in_maps(x, encoded_data, freqs, attn_bias, Wq, Wk, Wv, Wo,
                 q_scale, k_scale):
    import ml_dtypes
    consts = host_prep(np.asarray(freqs, np.float32),
                       np.asarray(q_scale, np.float32),
                       np.asarray(k_scale, np.float32))
    x = np.asarray(x, np.float32)
    e = np.asarray(encoded_data, np.float32)
    ab = np.asarray(attn_bias, np.float32)

    ii = np.arange(T)
    causal = ii[:, None] < ii[None, :]                       # (q, k): k > q
    ebias = np.where(causal[None], np.float32(-30.0), ab)    # (H, q, k)
    ebias = np.ascontiguousarray(ebias.transpose(0, 2, 1))   # (H, k, q)

    Wq = np.asarray(Wq, np.float32).reshape(C, H, D)[:, :, PERM]
    Wk = np.asarray(Wk, np.float32).reshape(C, KV, D)[:, :, PERM].reshape(C, KV * D)
    Wv = np.asarray(Wv, np.float32).reshape(C, KV, D)[:, :, PERM].reshape(C, KV * D)
    Wo = np.asarray(Wo, np.float32).reshape(H, D, C)[:, PERM, :]

    bf = ml_dtypes.bfloat16
    Wk_b = np.ascontiguousarray(Wk).astype(bf)
    Wv_b = np.ascontiguousarray(Wv).astype(bf)

    in_maps = []
    xT_c = {}
    eT_c = {}
    for core in range(8):
        b, g = core // 2, core % 2
        m = dict(consts)
        if b not in xT_c:
            xT_c[b] = np.ascontiguousarray(x[b].T).astype(bf)
            eT_c[b] = np.ascontiguousarray(e[b].T).astype(bf)
        m["xT"] = xT_c[b]
        m["eT"] = eT_c[b]
        m["wq"] = np.ascontiguousarray(
            Wq[:, g * HG:(g + 1) * HG].reshape(C, HG * D)).astype(bf)
        m["wk"] = Wk_b
        m["wv"] = Wv_b
        m["wo"] = np.ascontiguousarray(
            Wo[g * HG:(g + 1) * HG].reshape(HG * D, C)).astype(bf)
        p0, p1 = pack_ebias(ebias[g * HG:(g + 1) * HG])
        m["ebias0"] = p0
        m["ebias1"] = p1
        in_maps.append(m)
    return in_maps


def kernel(x, encoded_data, freqs, attn_bias, Wq, Wk, Wv, Wo,
           q_scale, k_scale):
    nc = get_nc()
    in_maps = make_in_maps(x, encoded_data, freqs, attn_bias,
                           Wq, Wk, Wv, Wo, q_scale, k_scale)
    res = run_bass_kernel_spmd(nc, in_maps, core_ids=list(range(8)))
    out = np.empty((B, T, C), np.float32)
    for b in range(B):
        out[b] = (res.results[2 * b]["out"].astype(np.float32)
                  + res.results[2 * b + 1]["out"].astype(np.float32))
    return out


# revision 8
# speedup vs baseline: 39.1852x; 1.8685x over previous
"""CrossAttention Trainium2 kernel (8-core SPMD).

Sharding: core c = (b, g) with b = c // 2 (batch), g = c % 2 (head group of 8).
Each core computes the full attention + partial output projection for its
(batch, 8-head group); the host sums the two partial o-proj results per batch.

Key design points (v2):
  - Host pre-transposes x/e to (C, T) and ships all matmul operands in bf16:
    projections consume xT/eT directly (no on-device PE transposes of x/e).
  - Rope dims are de-interleaved by a per-head permutation of the weight
    columns (scores are invariant under a shared q/k permutation; it is
    undone in Wo's rows for the v path), so all rope elementwise ops are
    contiguous and DVE-fast.
  - Host ships the causal-masked bias (-30 in masked positions, bf16),
    column-trimmed to the unmasked triangle and packed; fully-masked
    128-column blocks are skipped outright. Bias is added into the score
    PSUM via an identity matmul on PE (cheaper than a DVE multiply).
  - l2-norm + partial rotary in natural layout, then PE-transpose q/k (bf16);
    AV via lhsT = [V | ones] giving y^T and softmax denominators in one pass.
"""

import os
import sys
from contextlib import ExitStack

import numpy as np

if not os.path.isdir(os.path.join(os.path.dirname(os.path.abspath(__file__)), "concourse")):
    for _p in ("/opt/trn_rl_repo",):
        if os.path.isdir(_p) and _p not in sys.path:
            sys.path.insert(0, _p)

import concourse.bass as bass  # noqa: E402
import concourse.tile as tile  # noqa: E402
from concourse import bacc, mybir  # noqa: E402
from concourse.bass_utils import run_bass_kernel_spmd  # noqa: E402

B, T, C = 4, 1024, 1024
H, KV, D = 16, 8, 64
L = 32
HG = 8          # heads per group (= kv heads; local head l uses kv head l)
NG = 2          # head groups
QK_NORM_SCALE = 10.0
DS = float(D) ** -0.5
SCALE_Q = DS * DS / QK_NORM_SCALE   # folded into q's rsqrt(norm) factor

F32 = mybir.dt.float32
BF16 = mybir.dt.bfloat16

NT = T // 128   # 8 T-tiles
NC_ = C // 128  # 8 C-tiles

# packed, causal-trimmed exp(bias) widths: for query group qg, key tile kt,
# the needed query columns start at max(kt - qg*4, 0) * 128
def _col_layout(qg):
    offs, widths, qoffs = [], [], []
    o = 0
    for kt in range(qg * 4 + 4):
        i = kt - qg * 4
        w = 512 - max(i, 0) * 128
        offs.append(o)
        widths.append(w)
        qoffs.append(512 - w)
        o += w
    return offs, widths, qoffs, o

OFFS0, WID0, QOFF0, W0 = _col_layout(0)   # W0 = 1280
OFFS1, WID1, QOFF1, W1 = _col_layout(1)   # W1 = 3328

# one packed bf16 input blob per core, in DMA-priority order
_BLOB_SPEC = [
    ("identb", 128 * 128),
    ("xT", C * T),
    ("wq", C * HG * D),
    ("ropec", T * 288),
    ("eT", C * T),
    ("wk", C * KV * D),
    ("wv", C * KV * D),
    ("wo", HG * D * C),
    ("ebias0", HG * 128 * W0),
    ("ebias1", HG * 128 * W1),
]
BLOB_LAYOUT = {}
_off = 0
for _nm, _n in _BLOB_SPEC:
    BLOB_LAYOUT[_nm] = (_off, _n)
    _off += _n
BLOB_SIZE = _off


def build_program():
    nc = bacc.Bacc(
        "TRN2",
        target_bir_lowering=False,
        debug=False,
        enable_asserts=False,
        num_devices=8,
    )

    # All inputs are packed into one 1-D bf16 blob (fewer PJRT buffers =
    # cheaper per-call dispatch); BLOB_LAYOUT gives (name -> offset, size).
    blob = nc.dram_tensor("blob", (BLOB_SIZE,), BF16, kind="ExternalInput").ap()

    def bview(name):
        off, n = BLOB_LAYOUT[name]
        return blob[off:off + n]

    identb_d = bview("identb").rearrange("(a b) -> a b", b=128)
    xT = bview("xT")
    eT = bview("eT")
    wq = bview("wq")
    wk = bview("wk")
    wv = bview("wv")
    wo = bview("wo")
    ebias0 = bview("ebias0").rearrange("(h p w) -> h p w", p=128, w=W0)
    ebias1 = bview("ebias1").rearrange("(h p w) -> h p w", p=128, w=W1)
    ropec = bview("ropec")
    out_d = nc.dram_tensor("out", (T, C), BF16, kind="ExternalOutput").ap()

    with tile.TileContext(nc) as tc, ExitStack() as ctx:
        const = ctx.enter_context(tc.tile_pool(name="const", bufs=1))
        persist = ctx.enter_context(tc.tile_pool(name="persist", bufs=1))

        # ---- constants / weights (DMA order = priority order) ----
        identb = const.tile([128, 128], BF16, tag="identb")
        nc.sync.dma_start(identb[:], identb_d)

        xT_t = persist.tile([128, NC_ * T], BF16, tag="xT")
        xT_sb = xT_t.rearrange("p (cb t) -> p cb t", cb=NC_)
        nc.sync.dma_start(xT_sb, xT.rearrange("(cb p t) -> p cb t", p=128, t=T))

        wq_t = persist.tile([128, NC_ * 512], BF16, tag="wq")
        wq_sb = wq_t.rearrange("p (cb n) -> p cb n", cb=NC_)
        nc.sync.dma_start(wq_sb, wq.rearrange("(cb p n) -> p cb n", p=128, n=512))

        ropec_t = const.tile([128, NT * 288], BF16, tag="ropec")
        ropec_sb = ropec_t.rearrange("p (tt d) -> p tt d", tt=NT)
        nc.sync.dma_start(ropec_sb, ropec.rearrange("(tt p d) -> p tt d", p=128, d=288))

        # rope constant views: [cf(64) se(16) so(16)] x (q, k, v)
        def rviews(base):
            return (ropec_sb[:, :, base:base + 64],
                    ropec_sb[:, :, base + 64:base + 80],
                    ropec_sb[:, :, base + 80:base + 96])

        rope_q, rope_k, rope_v = rviews(0), rviews(96), rviews(192)

        # persistent: qT/kT (d on partitions, t free), va (t part, h x 65)
        qT = {(pl, hf): persist.tile([128, 512], BF16, tag=f"qT{pl}_{hf}",
                                     name=f"qT{pl}_{hf}")
              for pl in range(4) for hf in range(2)}
        kT = {(pl, hf): persist.tile([128, 512], BF16, tag=f"kT{pl}_{hf}",
                                     name=f"kT{pl}_{hf}")
              for pl in range(4) for hf in range(2)}
        va = [persist.tile([128, HG * 65], BF16, tag=f"va{tt}", name=f"va{tt}")
              for tt in range(NT)]

        # exp(bias) tiles, software-prefetched
        biasp = ctx.enter_context(tc.tile_pool(name="biasp", bufs=2))
        bias_tiles = {}

        def issue_bias(qg, lb):
            if (qg, lb) in bias_tiles:
                return
            W = W0 if qg == 0 else W1
            src = ebias0 if qg == 0 else ebias1
            bt = biasp.tile([128, 2 * W], BF16, tag=f"bias{qg}",
                            name=f"bias{qg}_{lb}")
            bt3 = bt.rearrange("p (h w) -> p h w", h=2)
            nc.sync.dma_start(bt3, src[lb:lb + 2].rearrange("h p w -> p h w"))
            bias_tiles[(qg, lb)] = bt3

        def rope_inplace(v3, tt, rope_views, smallp):
            """v3: (128, HG, 64) bf16 SBUF view, rope dims de-interleaved:
            new[0:16] = old even, new[16:32] = old odd, new[32:64] untouched
            by rotation (but cf applies the per-dim scale everywhere)."""
            cf, se, so = rope_views
            ev = v3[:, :, 0:16]
            od = v3[:, :, 16:32]
            se_b = se[:, tt].unsqueeze(1).broadcast_to([128, HG, 16])
            so_b = so[:, tt].unsqueeze(1).broadcast_to([128, HG, 16])
            cf_b = cf[:, tt].unsqueeze(1).broadcast_to([128, HG, D])
            tmp_e = smallp.tile([128, HG * 16], BF16, tag="tmpe", name="tmpe")
            tmp_o = smallp.tile([128, HG * 16], BF16, tag="tmpo", name="tmpo")
            te3 = tmp_e.rearrange("p (h d) -> p h d", h=HG)
            to3 = tmp_o.rearrange("p (h d) -> p h d", h=HG)
            nc.vector.tensor_mul(te3, od, se_b)
            nc.vector.tensor_mul(to3, ev, so_b)
            nc.vector.tensor_mul(v3[:, :, 0:D], v3[:, :, 0:D], cf_b)
            nc.vector.tensor_sub(ev, ev, te3)
            nc.vector.tensor_add(od, od, to3)

        def flush_qn(qns, ttg, tpsum, dstT):
            """PE-transpose 4 ready (128t, 512d) bf16 tiles into
            dstT[(pl, ttg)] (128d, 512t)."""
            for pl in range(4):
                ps4 = tpsum.tile([128, 512], BF16, tag="tps", name="tps")
                for tti in range(4):
                    nc.tensor.matmul(
                        ps4[:, tti * 128:(tti + 1) * 128],
                        qns[tti][:, pl * 128:(pl + 1) * 128],
                        identb[:], is_transpose=True, start=True, stop=True,
                    )
                nc.scalar.copy(dstT[(pl, ttg)][:], ps4[:])

        def norm_rope(ps, tt, which, smallp, sqp, rotp):
            """ps: (128 t, 512) psum of raw q/k projections. Per-head l2
            normalization (scaled), then rope; returns bf16 tile."""
            ps3 = ps.rearrange("p (h d) -> p h d", h=HG)
            sq = sqp.tile([128, HG * D], F32, tag="sq", name="sq")
            sq3 = sq.rearrange("p (h d) -> p h d", h=HG)
            nc.scalar.square(sq[:], ps[:])
            ss = smallp.tile([128, HG], F32, tag="ss", name="ss")
            nc.vector.tensor_reduce(
                ss[:], sq3, axis=mybir.AxisListType.X, op=mybir.AluOpType.add,
            )
            inv = smallp.tile([128, HG], F32, tag="inv", name="inv")
            nc.vector.reciprocal(inv[:], ss[:])
            rs = smallp.tile([128, HG], F32, tag="rs", name="rs")
            scl = SCALE_Q * SCALE_Q if which == "q" else 1.0
            nc.scalar.activation(
                rs[:], inv[:], mybir.ActivationFunctionType.Sqrt,
                bias=0.0, scale=scl,
            )
            qn = rotp.tile([128, HG * D], BF16, tag="qn", name="qn")
            d3 = qn.rearrange("p (h d) -> p h d", h=HG)
            nc.vector.tensor_mul(
                d3, ps3, rs[:].unsqueeze(2).broadcast_to([128, HG, D]),
            )
            rope_inplace(d3, tt, rope_q if which == "q" else rope_k, smallp)
            return qn

        # ---- projection phases ----
        with tc.tile_pool(name="projp", bufs=4, space="PSUM") as projp, \
             tc.tile_pool(name="tpsum", bufs=3, space="PSUM") as tpsum, \
             tc.tile_pool(name="smallp", bufs=6) as smallp, \
             tc.tile_pool(name="sqp", bufs=2) as sqp, \
             tc.tile_pool(name="rotp", bufs=5) as rotp:

            # phase X: Q = xT.T @ wq, per 128-row t-tile
            qns = []
            for tt in range(NT):
                ps = projp.tile([128, 512], F32, tag="proj", name="proj")
                for cb in range(NC_):
                    nc.tensor.matmul(
                        ps[:], xT_sb[:, cb, tt * 128:(tt + 1) * 128],
                        wq_sb[:, cb],
                        start=(cb == 0), stop=(cb == NC_ - 1),
                    )
                qns.append(norm_rope(ps, tt, "q", smallp, sqp, rotp))
                if tt % 4 == 3:
                    flush_qn(qns[-4:], tt // 4, tpsum, qT)
                if tt == 0:
                    # queue remaining input DMAs behind the x/wq loads
                    eT_t = persist.tile([128, NC_ * T], BF16, tag="eT")
                    eT_sb = eT_t.rearrange("p (cb t) -> p cb t", cb=NC_)
                    nc.sync.dma_start(
                        eT_sb, eT.rearrange("(cb p t) -> p cb t", p=128, t=T))
                    wk_t = persist.tile([128, NC_ * 512], BF16, tag="wk")
                    wk_sb = wk_t.rearrange("p (cb n) -> p cb n", cb=NC_)
                    nc.sync.dma_start(
                        wk_sb, wk.rearrange("(cb p n) -> p cb n", p=128, n=512))
                    wv_t = persist.tile([128, NC_ * 512], BF16, tag="wv")
                    wv_sb = wv_t.rearrange("p (cb n) -> p cb n", cb=NC_)
                    nc.sync.dma_start(
                        wv_sb, wv.rearrange("(cb p n) -> p cb n", p=128, n=512))
                    wo_t = persist.tile([128, 4 * C], BF16, tag="wo")
                    wo_sb = wo_t.rearrange("p (pl c) -> p pl c", pl=4)
                    nc.sync.dma_start(
                        wo_sb, wo.rearrange("(pl p c) -> p pl c", p=128, c=C))
                    issue_bias(0, 0)
                    issue_bias(0, 2)

            # phase E: K and V from eT
            kns = []
            for tt in range(NT):
                ps = projp.tile([128, 512], F32, tag="proj", name="proj")
                for cb in range(NC_):
                    nc.tensor.matmul(
                        ps[:], eT_sb[:, cb, tt * 128:(tt + 1) * 128],
                        wk_sb[:, cb],
                        start=(cb == 0), stop=(cb == NC_ - 1),
                    )
                kns.append(norm_rope(ps, tt, "k", smallp, sqp, rotp))
                if tt % 4 == 3:
                    flush_qn(kns[-4:], tt // 4, tpsum, kT)
                psv = projp.tile([128, 512], F32, tag="proj", name="projv")
                for cb in range(NC_):
                    nc.tensor.matmul(
                        psv[:], eT_sb[:, cb, tt * 128:(tt + 1) * 128],
                        wv_sb[:, cb],
                        start=(cb == 0), stop=(cb == NC_ - 1),
                    )
                v3 = va[tt].rearrange("p (h e) -> p h e", h=HG)
                nc.vector.tensor_copy(
                    v3[:, :, 0:D], psv.rearrange("p (h d) -> p h d", h=HG))
                nc.vector.memset(v3[:, :, D:D + 1], 1.0)
                rope_inplace(v3, tt, rope_v, smallp)

        # ---- attention (qg-outer) + interleaved o-proj ----
        ys = {}
        for pl in range(4):
            for qg in range(2):
                ys[(pl, qg)] = persist.tile([128, 512], BF16,
                                            tag=f"ys{pl}_{qg}",
                                            name=f"ys{pl}_{qg}")

        with tc.tile_pool(name="attp", bufs=6) as attp, \
             tc.tile_pool(name="spsum", bufs=4, space="PSUM") as spsum, \
             tc.tile_pool(name="ypsum", bufs=2, space="PSUM") as ypsum, \
             tc.tile_pool(name="opsum", bufs=2, space="PSUM") as opsum, \
             tc.tile_pool(name="outp", bufs=2) as outp, \
             tc.tile_pool(name="smalle", bufs=4) as smalle:

            def oproj(tt):
                ot = outp.tile([128, C], BF16, tag="ot", name="ot")
                qg = tt // 4
                for cg in range(2):
                    pso = opsum.tile([128, 512], F32, tag="pso", name="pso")
                    for pl in range(4):
                        nc.tensor.matmul(
                            pso[:],
                            ys[(pl, qg)][:, (tt % 4) * 128:(tt % 4 + 1) * 128],
                            wo_sb[:, pl, cg * 512:(cg + 1) * 512],
                            start=(pl == 0), stop=(pl == 3),
                        )
                    nc.vector.tensor_copy(ot[:, cg * 512:(cg + 1) * 512], pso[:])
                nc.sync.dma_start(out_d[tt * 128:(tt + 1) * 128, :], ot[:])

            for qg in range(2):
                nkt = qg * 4 + 4
                offs = OFFS0 if qg == 0 else OFFS1
                wids = WID0 if qg == 0 else WID1
                qoffs = QOFF0 if qg == 0 else QOFF1
                for lb in range(0, HG, 2):      # head blocks of 2
                    issue_bias(qg, lb)
                    # prefetch the block after next (bufs=2 per qg tag)
                    nlb = lb + 2
                    if nlb < HG:
                        issue_bias(qg, nlb)
                    elif qg == 0:
                        issue_bias(1, 0)
                    bt3 = bias_tiles[(qg, lb)]
                    for l4 in range(2):
                        l = lb + l4
                        pl, sub = l // 2, l % 2
                        po = 64 * sub
                        psy = ypsum.tile([65, 512], F32, tag="psy", name="psy")
                        for kt in range(nkt):
                            w, qo, off = wids[kt], qoffs[kt], offs[kt]
                            pss = spsum.tile([128, 512], F32, tag="pss",
                                             name="pss")
                            nc.tensor.matmul(
                                pss[:, 0:w],
                                kT[(pl, kt // 4)][po:po + 64,
                                                  (kt % 4) * 128:(kt % 4 + 1) * 128],
                                qT[(pl, qg)][po:po + 64, qo:512],
                                start=True, stop=False,
                            )
                            nc.tensor.matmul(
                                pss[:, 0:w], identb[:],
                                bt3[:, l4, off:off + w],
                                start=False, stop=True,
                            )
                            att = attp.tile([128, 512], BF16, tag="att",
                                            name="att")
                            nc.scalar.activation(
                                att[:, 0:w], pss[:, 0:w],
                                mybir.ActivationFunctionType.Exp,
                            )
                            nc.tensor.matmul(
                                psy[:, qo:512],
                                va[kt][:, l * 65:(l + 1) * 65],
                                att[:, 0:w],
                                start=(kt == 0), stop=(kt == nkt - 1),
                            )
                        rcp = smalle.tile([1, 512], F32, tag="rcp", name="rcp")
                        nc.vector.reciprocal(rcp[:], psy[64:65, :])
                        rb = smalle.tile([64, 512], F32, tag="rb", name="rb")
                        nc.gpsimd.partition_broadcast(rb[:], rcp[:])
                        nc.vector.tensor_mul(
                            ys[(pl, qg)][po:po + 64, :],
                            psy[0:64, :], rb[:],
                        )
                # after all heads of this qg: o-proj for its 4 Tq tiles
                for tt in range(qg * 4, qg * 4 + 4):
                    oproj(tt)

    nc.compile()
    return nc


PERM = np.concatenate([np.arange(0, L, 2), np.arange(1, L, 2),
                       np.arange(L, D)])   # de-interleave rope pairs


def host_prep(freqs, q_scale, k_scale):
    """Rope constant tensors in the permuted dim order, packed (T, 288)."""
    c = np.cos(freqs[:, 0::2]).astype(np.float32)   # (T, 16)
    s = np.sin(freqs[:, 0::2]).astype(np.float32)
    import ml_dtypes
    ropec = np.empty((T, 288), np.float32)
    for j, scale in enumerate((q_scale, k_scale, np.ones(D, np.float32))):
        scale = np.asarray(scale, np.float32)
        cf = np.empty((T, D), np.float32)
        cf[:, 0:L:2] = c * scale[0:L:2][None, :]
        cf[:, 1:L:2] = c * scale[1:L:2][None, :]
        cf[:, L:] = scale[L:][None, :]
        se = s * scale[1:L:2][None, :]    # multiplies odd input -> even out
        so = s * scale[0:L:2][None, :]    # multiplies even input -> odd out
        base = j * 96
        ropec[:, base:base + 64] = cf[:, PERM]
        ropec[:, base + 64:base + 80] = se
        ropec[:, base + 80:base + 96] = so
    consts = {
        "ropec": ropec.astype(ml_dtypes.bfloat16),
        "identb": np.eye(128, dtype=ml_dtypes.bfloat16),
    }
    return consts


def pack_ebias(eb_g):
    """eb_g: (HG, Tk, Tq) float32 exp(bias) with causal zeros, for one head
    group. Returns packed (HG, 128, W0), (HG, 128, W1) bf16."""
    import ml_dtypes
    p0 = np.empty((HG, 128, W0), np.float32)
    p1 = np.empty((HG, 128, W1), np.float32)
    for qg, (p, offs, wids, qoffs) in enumerate(
            ((p0, OFFS0, WID0, QOFF0), (p1, OFFS1, WID1, QOFF1))):
        q0 = qg * 512
        for kt in range(qg * 4 + 4):
            o, w, qo = offs[kt], wids[kt], qoffs[kt]
            p[:, :, o:o + w] = eb_g[:, kt * 128:(kt + 1) * 128,
                                    q0 + qo:q0 + 512]
    return p0.astype(ml_dtypes.bfloat16), p1.astype(ml_dtypes.bfloat16)


_NC_CACHE = {}


def get_nc():
    if "nc" not in _NC_CACHE:
        _NC_CACHE["nc"] = build_program()
    return _NC_CACHE["nc"]


def make_in_maps(x, encoded_data, freqs, attn_bias, Wq, Wk, Wv, Wo,
                 q_scale, k_scale):
    import ml_dtypes
    consts = host_prep(np.asarray(freqs, np.float32),
                       np.asarray(q_scale, np.float32),
                       np.asarray(k_scale, np.float32))
    x = np.asarray(x, np.float32)
    e = np.asarray(encoded_data, np.float32)
    ab = np.asarray(attn_bias, np.float32)

    ii = np.arange(T)
    causal = ii[:, None] < ii[None, :]                       # (q, k): k > q
    ebias = np.where(causal[None], np.float32(-30.0), ab)    # (H, q, k)
    ebias = np.ascontiguousarray(ebias.transpose(0, 2, 1))   # (H, k, q)

    Wq = np.asarray(Wq, np.float32).reshape(C, H, D)[:, :, PERM]
    Wk = np.asarray(Wk, np.float32).reshape(C, KV, D)[:, :, PERM].reshape(C, KV * D)
    Wv = np.asarray(Wv, np.float32).reshape(C, KV, D)[:, :, PERM].reshape(C, KV * D)
    Wo = np.asarray(Wo, np.float32).reshape(H, D, C)[:, PERM, :]

    bf = ml_dtypes.bfloat16
    Wk_b = np.ascontiguousarray(Wk).astype(bf)
    Wv_b = np.ascontiguousarray(Wv).astype(bf)

    in_maps = []
    xT_c = {}
    eT_c = {}
    for core in range(8):
        b, g = core // 2, core % 2
        parts = dict(consts)
        if b not in xT_c:
            xT_c[b] = np.ascontiguousarray(x[b].T).astype(bf)
            eT_c[b] = np.ascontiguousarray(e[b].T).astype(bf)
        parts["xT"] = xT_c[b]
        parts["eT"] = eT_c[b]
        parts["wq"] = np.ascontiguousarray(
            Wq[:, g * HG:(g + 1) * HG].reshape(C, HG * D)).astype(bf)
        parts["wk"] = Wk_b
        parts["wv"] = Wv_b
        parts["wo"] = np.ascontiguousarray(
            Wo[g * HG:(g + 1) * HG].reshape(HG * D, C)).astype(bf)
        p0, p1 = pack_ebias(ebias[g * HG:(g + 1) * HG])
        parts["ebias0"] = p0
        parts["ebias1"] = p1
        blob = np.empty((BLOB_SIZE,), bf)
        for nm, n in _BLOB_SPEC:
            off, _ = BLOB_LAYOUT[nm]
            blob[off:off + n] = np.ascontiguousarray(parts[nm]).reshape(-1)
        in_maps.append({"blob": blob})
    return in_maps


def kernel(x, encoded_data, freqs, attn_bias, Wq, Wk, Wv, Wo,
           q_scale, k_scale):
    nc = get_nc()
    in_maps = make_in_maps(x, encoded_data, freqs, attn_bias,
                           Wq, Wk, Wv, Wo, q_scale, k_scale)
    res = run_bass_kernel_spmd(nc, in_maps, core_ids=list(range(8)))
    out = np.empty((B, T, C), np.float32)
    for b in range(B):
        out[b] = (res.results[2 * b]["out"].astype(np.float32)
                  + res.results[2 * b + 1]["out"].astype(np.float32))
    return out


# revision 9
# speedup vs baseline: 62.2307x; 1.5881x over previous
"""CrossAttention Trainium2 kernel (8-core SPMD).

Sharding: core c = (b, g) with b = c // 2 (batch), g = c % 2 (head group of 8).
Each core computes the full attention + partial output projection for its
(batch, 8-head group); the host sums the two partial o-proj results per batch.

Key design points (v2):
  - Host pre-transposes x/e to (C, T) and ships all matmul operands in bf16:
    projections consume xT/eT directly (no on-device PE transposes of x/e).
  - Rope dims are de-interleaved by a per-head permutation of the weight
    columns (scores are invariant under a shared q/k permutation; it is
    undone in Wo's rows for the v path), so all rope elementwise ops are
    contiguous and DVE-fast.
  - Host ships the causal-masked bias (-30 in masked positions, bf16),
    column-trimmed to the unmasked triangle and packed; fully-masked
    128-column blocks are skipped outright. Bias is added into the score
    PSUM via an identity matmul on PE (cheaper than a DVE multiply).
  - l2-norm + partial rotary in natural layout, then PE-transpose q/k (bf16);
    AV via lhsT = [V | ones] giving y^T and softmax denominators in one pass.
"""

import os
import sys
from contextlib import ExitStack

import numpy as np

if not os.path.isdir(os.path.join(os.path.dirname(os.path.abspath(__file__)), "concourse")):
    for _p in ("/opt/trn_rl_repo",):
        if os.path.isdir(_p) and _p not in sys.path:
            sys.path.insert(0, _p)

import concourse.bass as bass  # noqa: E402
import concourse.tile as tile  # noqa: E402
from concourse import bacc, mybir  # noqa: E402
from concourse.bass_utils import run_bass_kernel_spmd  # noqa: E402

B, T, C = 4, 1024, 1024
H, KV, D = 16, 8, 64
L = 32
HG = 8          # heads per group (= kv heads; local head l uses kv head l)
NG = 2          # head groups
QK_NORM_SCALE = 10.0
DS = float(D) ** -0.5
SCALE_Q = DS * DS / QK_NORM_SCALE   # folded into q's rsqrt(norm) factor

F32 = mybir.dt.float32
BF16 = mybir.dt.bfloat16

NT = T // 128   # 8 T-tiles
NC_ = C // 128  # 8 C-tiles

# packed, causal-trimmed exp(bias) widths: for query group qg, key tile kt,
# the needed query columns start at max(kt - qg*4, 0) * 128
def _col_layout(qg):
    offs, widths, qoffs = [], [], []
    o = 0
    for kt in range(qg * 4 + 4):
        i = kt - qg * 4
        w = 512 - max(i, 0) * 128
        offs.append(o)
        widths.append(w)
        qoffs.append(512 - w)
        o += w
    return offs, widths, qoffs, o

OFFS0, WID0, QOFF0, W0 = _col_layout(0)   # W0 = 1280
OFFS1, WID1, QOFF1, W1 = _col_layout(1)   # W1 = 3328

# one packed bf16 input blob per core, in DMA-priority order
_BLOB_SPEC = [
    ("identb", 128 * 128),
    ("xT", C * T),
    ("wq", C * HG * D),
    ("ropec", T * 288),
    ("eT", C * T),
    ("wk", C * KV * D),
    ("wv", C * KV * D),
    ("wo", HG * D * C),
    ("ebias0", HG * 128 * W0),
    ("ebias1", HG * 128 * W1),
]
BLOB_LAYOUT = {}
_off = 0
for _nm, _n in _BLOB_SPEC:
    BLOB_LAYOUT[_nm] = (_off, _n)
    _off += _n
BLOB_SIZE = _off


def build_program():
    nc = bacc.Bacc(
        "TRN2",
        target_bir_lowering=False,
        debug=False,
        enable_asserts=False,
        num_devices=8,
    )

    # All inputs are packed into one 1-D bf16 blob (fewer PJRT buffers =
    # cheaper per-call dispatch); BLOB_LAYOUT gives (name -> offset, size).
    blob = nc.dram_tensor("blob", (BLOB_SIZE,), BF16, kind="ExternalInput").ap()

    def bview(name):
        off, n = BLOB_LAYOUT[name]
        return blob[off:off + n]

    identb_d = bview("identb").rearrange("(a b) -> a b", b=128)
    xT = bview("xT")
    eT = bview("eT")
    wq = bview("wq")
    wk = bview("wk")
    wv = bview("wv")
    wo = bview("wo")
    ebias0 = bview("ebias0").rearrange("(h p w) -> h p w", p=128, w=W0)
    ebias1 = bview("ebias1").rearrange("(h p w) -> h p w", p=128, w=W1)
    ropec = bview("ropec")
    out_d = nc.dram_tensor("out", (T, C), BF16, kind="ExternalOutput").ap()

    with tile.TileContext(nc) as tc, ExitStack() as ctx:
        const = ctx.enter_context(tc.tile_pool(name="const", bufs=1))
        persist = ctx.enter_context(tc.tile_pool(name="persist", bufs=1))

        # ---- constants / weights (DMA order = priority order) ----
        identb = const.tile([128, 128], BF16, tag="identb")
        nc.sync.dma_start(identb[:], identb_d)

        xT_t = persist.tile([128, NC_ * T], BF16, tag="xT")
        xT_sb = xT_t.rearrange("p (cb t) -> p cb t", cb=NC_)
        nc.sync.dma_start(xT_sb, xT.rearrange("(cb p t) -> p cb t", p=128, t=T))

        wq_t = persist.tile([128, NC_ * 512], BF16, tag="wq")
        wq_sb = wq_t.rearrange("p (cb n) -> p cb n", cb=NC_)
        nc.sync.dma_start(wq_sb, wq.rearrange("(cb p n) -> p cb n", p=128, n=512))

        ropec_t = const.tile([128, NT * 288], BF16, tag="ropec")
        ropec_sb = ropec_t.rearrange("p (tt d) -> p tt d", tt=NT)
        nc.sync.dma_start(ropec_sb, ropec.rearrange("(tt p d) -> p tt d", p=128, d=288))

        # rope constant views: [cf(64) se(16) so(16)] x (q, k, v)
        def rviews(base):
            return (ropec_sb[:, :, base:base + 64],
                    ropec_sb[:, :, base + 64:base + 80],
                    ropec_sb[:, :, base + 80:base + 96])

        rope_q, rope_k, rope_v = rviews(0), rviews(96), rviews(192)

        # persistent: qT/kT (d on partitions, t free), va (t part, h x 65)
        qT = {(pl, hf): persist.tile([128, 512], BF16, tag=f"qT{pl}_{hf}",
                                     name=f"qT{pl}_{hf}")
              for pl in range(4) for hf in range(2)}
        kT = {(pl, hf): persist.tile([128, 512], BF16, tag=f"kT{pl}_{hf}",
                                     name=f"kT{pl}_{hf}")
              for pl in range(4) for hf in range(2)}
        va = [persist.tile([128, HG * 65], BF16, tag=f"va{tt}", name=f"va{tt}")
              for tt in range(NT)]

        # exp(bias) tiles, software-prefetched
        biasp = ctx.enter_context(tc.tile_pool(name="biasp", bufs=2))
        bias_tiles = {}

        def issue_bias(qg, lb):
            if (qg, lb) in bias_tiles:
                return
            W = W0 if qg == 0 else W1
            src = ebias0 if qg == 0 else ebias1
            bt = biasp.tile([128, 2 * W], BF16, tag=f"bias{qg}",
                            name=f"bias{qg}_{lb}")
            bt3 = bt.rearrange("p (h w) -> p h w", h=2)
            nc.sync.dma_start(bt3, src[lb:lb + 2].rearrange("h p w -> p h w"))
            bias_tiles[(qg, lb)] = bt3

        def rope_inplace(v3, tt, rope_views, smallp):
            """v3: (128, HG, 64) bf16 SBUF view, rope dims de-interleaved:
            new[0:16] = old even, new[16:32] = old odd, new[32:64] untouched
            by rotation (but cf applies the per-dim scale everywhere)."""
            cf, se, so = rope_views
            ev = v3[:, :, 0:16]
            od = v3[:, :, 16:32]
            se_b = se[:, tt].unsqueeze(1).broadcast_to([128, HG, 16])
            so_b = so[:, tt].unsqueeze(1).broadcast_to([128, HG, 16])
            cf_b = cf[:, tt].unsqueeze(1).broadcast_to([128, HG, D])
            tmp_e = smallp.tile([128, HG * 16], BF16, tag="tmpe", name="tmpe")
            tmp_o = smallp.tile([128, HG * 16], BF16, tag="tmpo", name="tmpo")
            te3 = tmp_e.rearrange("p (h d) -> p h d", h=HG)
            to3 = tmp_o.rearrange("p (h d) -> p h d", h=HG)
            nc.vector.tensor_mul(te3, od, se_b)
            nc.vector.tensor_mul(to3, ev, so_b)
            nc.vector.tensor_mul(v3[:, :, 0:D], v3[:, :, 0:D], cf_b)
            nc.vector.tensor_sub(ev, ev, te3)
            nc.vector.tensor_add(od, od, to3)

        def flush_qn(qns, ttg, tpsum, dstT):
            """PE-transpose 4 ready (128t, 512d) bf16 tiles into
            dstT[(pl, ttg)] (128d, 512t)."""
            for pl in range(4):
                ps4 = tpsum.tile([128, 512], BF16, tag="tps", name="tps")
                for tti in range(4):
                    nc.tensor.matmul(
                        ps4[:, tti * 128:(tti + 1) * 128],
                        qns[tti][:, pl * 128:(pl + 1) * 128],
                        identb[:], is_transpose=True, start=True, stop=True,
                    )
                nc.scalar.copy(dstT[(pl, ttg)][:], ps4[:])

        def norm_rope(ps, tt, which, smallp, sqp, rotp):
            """ps: (128 t, 512) psum of raw q/k projections. Per-head l2
            normalization (scaled), then rope; returns bf16 tile."""
            ps3 = ps.rearrange("p (h d) -> p h d", h=HG)
            sq = sqp.tile([128, HG * D], F32, tag="sq", name="sq")
            sq3 = sq.rearrange("p (h d) -> p h d", h=HG)
            nc.scalar.square(sq[:], ps[:])
            ss = smallp.tile([128, HG], F32, tag="ss", name="ss")
            nc.vector.tensor_reduce(
                ss[:], sq3, axis=mybir.AxisListType.X, op=mybir.AluOpType.add,
            )
            inv = smallp.tile([128, HG], F32, tag="inv", name="inv")
            nc.vector.reciprocal(inv[:], ss[:])
            rs = smallp.tile([128, HG], F32, tag="rs", name="rs")
            scl = SCALE_Q * SCALE_Q if which == "q" else 1.0
            nc.scalar.activation(
                rs[:], inv[:], mybir.ActivationFunctionType.Sqrt,
                bias=0.0, scale=scl,
            )
            qn = rotp.tile([128, HG * D], BF16, tag="qn", name="qn")
            d3 = qn.rearrange("p (h d) -> p h d", h=HG)
            nc.vector.tensor_mul(
                d3, ps3, rs[:].unsqueeze(2).broadcast_to([128, HG, D]),
            )
            rope_inplace(d3, tt, rope_q if which == "q" else rope_k, smallp)
            return qn

        # ---- projection phases ----
        with tc.tile_pool(name="projp", bufs=4, space="PSUM") as projp, \
             tc.tile_pool(name="tpsum", bufs=3, space="PSUM") as tpsum, \
             tc.tile_pool(name="smallp", bufs=6) as smallp, \
             tc.tile_pool(name="sqp", bufs=2) as sqp, \
             tc.tile_pool(name="rotp", bufs=5) as rotp:

            # phase X: Q = xT.T @ wq, per 128-row t-tile
            qns = []
            for tt in range(NT):
                ps = projp.tile([128, 512], F32, tag="proj", name="proj")
                for cb in range(NC_):
                    nc.tensor.matmul(
                        ps[:], xT_sb[:, cb, tt * 128:(tt + 1) * 128],
                        wq_sb[:, cb],
                        start=(cb == 0), stop=(cb == NC_ - 1),
                    )
                qns.append(norm_rope(ps, tt, "q", smallp, sqp, rotp))
                if tt % 4 == 3:
                    flush_qn(qns[-4:], tt // 4, tpsum, qT)
                if tt == 0:
                    # queue remaining input DMAs behind the x/wq loads
                    eT_t = persist.tile([128, NC_ * T], BF16, tag="eT")
                    eT_sb = eT_t.rearrange("p (cb t) -> p cb t", cb=NC_)
                    nc.sync.dma_start(
                        eT_sb, eT.rearrange("(cb p t) -> p cb t", p=128, t=T))
                    wk_t = persist.tile([128, NC_ * 512], BF16, tag="wk")
                    wk_sb = wk_t.rearrange("p (cb n) -> p cb n", cb=NC_)
                    nc.sync.dma_start(
                        wk_sb, wk.rearrange("(cb p n) -> p cb n", p=128, n=512))
                    wv_t = persist.tile([128, NC_ * 512], BF16, tag="wv")
                    wv_sb = wv_t.rearrange("p (cb n) -> p cb n", cb=NC_)
                    nc.sync.dma_start(
                        wv_sb, wv.rearrange("(cb p n) -> p cb n", p=128, n=512))
                    wo_t = persist.tile([128, 4 * C], BF16, tag="wo")
                    wo_sb = wo_t.rearrange("p (pl c) -> p pl c", pl=4)
                    nc.sync.dma_start(
                        wo_sb, wo.rearrange("(pl p c) -> p pl c", p=128, c=C))
                    issue_bias(0, 0)
                    issue_bias(0, 2)

            # phase E: K and V from eT
            kns = []
            for tt in range(NT):
                ps = projp.tile([128, 512], F32, tag="proj", name="proj")
                for cb in range(NC_):
                    nc.tensor.matmul(
                        ps[:], eT_sb[:, cb, tt * 128:(tt + 1) * 128],
                        wk_sb[:, cb],
                        start=(cb == 0), stop=(cb == NC_ - 1),
                    )
                kns.append(norm_rope(ps, tt, "k", smallp, sqp, rotp))
                if tt % 4 == 3:
                    flush_qn(kns[-4:], tt // 4, tpsum, kT)
                psv = projp.tile([128, 512], F32, tag="proj", name="projv")
                for cb in range(NC_):
                    nc.tensor.matmul(
                        psv[:], eT_sb[:, cb, tt * 128:(tt + 1) * 128],
                        wv_sb[:, cb],
                        start=(cb == 0), stop=(cb == NC_ - 1),
                    )
                v3 = va[tt].rearrange("p (h e) -> p h e", h=HG)
                nc.vector.tensor_copy(
                    v3[:, :, 0:D], psv.rearrange("p (h d) -> p h d", h=HG))
                nc.vector.memset(v3[:, :, D:D + 1], 1.0)
                rope_inplace(v3, tt, rope_v, smallp)

        # ---- attention (qg-outer) + interleaved o-proj ----
        ys = {}
        for pl in range(4):
            for qg in range(2):
                ys[(pl, qg)] = persist.tile([128, 512], BF16,
                                            tag=f"ys{pl}_{qg}",
                                            name=f"ys{pl}_{qg}")

        with tc.tile_pool(name="attp", bufs=6) as attp, \
             tc.tile_pool(name="spsum", bufs=4, space="PSUM") as spsum, \
             tc.tile_pool(name="ypsum", bufs=2, space="PSUM") as ypsum, \
             tc.tile_pool(name="opsum", bufs=2, space="PSUM") as opsum, \
             tc.tile_pool(name="outp", bufs=2) as outp, \
             tc.tile_pool(name="smalle", bufs=4) as smalle:

            def oproj(tt):
                ot = outp.tile([128, C], BF16, tag="ot", name="ot")
                qg = tt // 4
                for cg in range(2):
                    pso = opsum.tile([128, 512], F32, tag="pso", name="pso")
                    for pl in range(4):
                        nc.tensor.matmul(
                            pso[:],
                            ys[(pl, qg)][:, (tt % 4) * 128:(tt % 4 + 1) * 128],
                            wo_sb[:, pl, cg * 512:(cg + 1) * 512],
                            start=(pl == 0), stop=(pl == 3),
                        )
                    nc.vector.tensor_copy(ot[:, cg * 512:(cg + 1) * 512], pso[:])
                nc.sync.dma_start(out_d[tt * 128:(tt + 1) * 128, :], ot[:])

            for qg in range(2):
                nkt = qg * 4 + 4
                offs = OFFS0 if qg == 0 else OFFS1
                wids = WID0 if qg == 0 else WID1
                qoffs = QOFF0 if qg == 0 else QOFF1
                for lb in range(0, HG, 2):      # head blocks of 2
                    issue_bias(qg, lb)
                    # prefetch the block after next (bufs=2 per qg tag)
                    nlb = lb + 2
                    if nlb < HG:
                        issue_bias(qg, nlb)
                    elif qg == 0:
                        issue_bias(1, 0)
                    bt3 = bias_tiles[(qg, lb)]
                    for l4 in range(2):
                        l = lb + l4
                        pl, sub = l // 2, l % 2
                        po = 64 * sub
                        psy = ypsum.tile([65, 512], F32, tag="psy", name="psy")
                        for kt in range(nkt):
                            w, qo, off = wids[kt], qoffs[kt], offs[kt]
                            pss = spsum.tile([128, 512], F32, tag="pss",
                                             name="pss")
                            nc.tensor.matmul(
                                pss[:, 0:w],
                                kT[(pl, kt // 4)][po:po + 64,
                                                  (kt % 4) * 128:(kt % 4 + 1) * 128],
                                qT[(pl, qg)][po:po + 64, qo:512],
                                start=True, stop=False,
                            )
                            nc.tensor.matmul(
                                pss[:, 0:w], identb[:],
                                bt3[:, l4, off:off + w],
                                start=False, stop=True,
                            )
                            att = attp.tile([128, 512], BF16, tag="att",
                                            name="att")
                            nc.scalar.activation(
                                att[:, 0:w], pss[:, 0:w],
                                mybir.ActivationFunctionType.Exp,
                            )
                            nc.tensor.matmul(
                                psy[:, qo:512],
                                va[kt][:, l * 65:(l + 1) * 65],
                                att[:, 0:w],
                                start=(kt == 0), stop=(kt == nkt - 1),
                            )
                        rcp = smalle.tile([1, 512], F32, tag="rcp", name="rcp")
                        nc.vector.reciprocal(rcp[:], psy[64:65, :])
                        rb = smalle.tile([64, 512], F32, tag="rb", name="rb")
                        nc.gpsimd.partition_broadcast(rb[:], rcp[:])
                        nc.vector.tensor_mul(
                            ys[(pl, qg)][po:po + 64, :],
                            psy[0:64, :], rb[:],
                        )
                # after all heads of this qg: o-proj for its 4 Tq tiles
                for tt in range(qg * 4, qg * 4 + 4):
                    oproj(tt)

    nc.compile()
    return nc


PERM = np.concatenate([np.arange(0, L, 2), np.arange(1, L, 2),
                       np.arange(L, D)])   # de-interleave rope pairs


def host_prep(freqs, q_scale, k_scale):
    """Rope constant tensors in the permuted dim order, packed (T, 288)."""
    c = np.cos(freqs[:, 0::2]).astype(np.float32)   # (T, 16)
    s = np.sin(freqs[:, 0::2]).astype(np.float32)
    import ml_dtypes
    ropec = np.empty((T, 288), np.float32)
    for j, scale in enumerate((q_scale, k_scale, np.ones(D, np.float32))):
        scale = np.asarray(scale, np.float32)
        cf = np.empty((T, D), np.float32)
        cf[:, 0:L:2] = c * scale[0:L:2][None, :]
        cf[:, 1:L:2] = c * scale[1:L:2][None, :]
        cf[:, L:] = scale[L:][None, :]
        se = s * scale[1:L:2][None, :]    # multiplies odd input -> even out
        so = s * scale[0:L:2][None, :]    # multiplies even input -> odd out
        base = j * 96
        ropec[:, base:base + 64] = cf[:, PERM]
        ropec[:, base + 64:base + 80] = se
        ropec[:, base + 80:base + 96] = so
    consts = {
        "ropec": ropec.astype(ml_dtypes.bfloat16),
        "identb": np.eye(128, dtype=ml_dtypes.bfloat16),
    }
    return consts


def pack_ebias(eb_g):
    """eb_g: (HG, Tk, Tq) float32 exp(bias) with causal zeros, for one head
    group. Returns packed (HG, 128, W0), (HG, 128, W1) bf16."""
    import ml_dtypes
    p0 = np.empty((HG, 128, W0), np.float32)
    p1 = np.empty((HG, 128, W1), np.float32)
    for qg, (p, offs, wids, qoffs) in enumerate(
            ((p0, OFFS0, WID0, QOFF0), (p1, OFFS1, WID1, QOFF1))):
        q0 = qg * 512
        for kt in range(qg * 4 + 4):
            o, w, qo = offs[kt], wids[kt], qoffs[kt]
            p[:, :, o:o + w] = eb_g[:, kt * 128:(kt + 1) * 128,
                                    q0 + qo:q0 + 512]
    return p0.astype(ml_dtypes.bfloat16), p1.astype(ml_dtypes.bfloat16)


_NC_CACHE = {}


def get_nc():
    if "nc" not in _NC_CACHE:
        _NC_CACHE["nc"] = build_program()
    return _NC_CACHE["nc"]


def make_in_maps(x, encoded_data, freqs, attn_bias, Wq, Wk, Wv, Wo,
                 q_scale, k_scale):
    import ml_dtypes
    consts = host_prep(np.asarray(freqs, np.float32),
                       np.asarray(q_scale, np.float32),
                       np.asarray(k_scale, np.float32))
    x = np.asarray(x, np.float32)
    e = np.asarray(encoded_data, np.float32)
    ab = np.asarray(attn_bias, np.float32)

    ii = np.arange(T)
    causal = ii[:, None] < ii[None, :]                       # (q, k): k > q
    ebias = np.where(causal[None], np.float32(-30.0), ab)    # (H, q, k)
    ebias = np.ascontiguousarray(ebias.transpose(0, 2, 1))   # (H, k, q)

    Wq = np.asarray(Wq, np.float32).reshape(C, H, D)[:, :, PERM]
    Wk = np.asarray(Wk, np.float32).reshape(C, KV, D)[:, :, PERM].reshape(C, KV * D)
    Wv = np.asarray(Wv, np.float32).reshape(C, KV, D)[:, :, PERM].reshape(C, KV * D)
    Wo = np.asarray(Wo, np.float32).reshape(H, D, C)[:, PERM, :]

    bf = ml_dtypes.bfloat16
    Wk_b = np.ascontiguousarray(Wk).astype(bf)
    Wv_b = np.ascontiguousarray(Wv).astype(bf)

    # per-b and per-g parts are cached: cores (b, 0/1) share xT/eT, cores
    # (0..3, g) share weights and the packed bias
    xT_c, eT_c, g_parts = {}, {}, {}
    for b in range(B):
        xT_c[b] = np.ascontiguousarray(x[b].T).astype(bf)
        eT_c[b] = np.ascontiguousarray(e[b].T).astype(bf)
    for g in range(NG):
        p0, p1 = pack_ebias(ebias[g * HG:(g + 1) * HG])
        g_parts[g] = {
            "wq": np.ascontiguousarray(
                Wq[:, g * HG:(g + 1) * HG].reshape(C, HG * D)).astype(bf),
            "wk": Wk_b,
            "wv": Wv_b,
            "wo": np.ascontiguousarray(
                Wo[g * HG:(g + 1) * HG].reshape(HG * D, C)).astype(bf),
            "ebias0": p0,
            "ebias1": p1,
        }
    in_maps = []
    for core in range(8):
        b, g = core // 2, core % 2
        parts = dict(consts)
        parts["xT"] = xT_c[b]
        parts["eT"] = eT_c[b]
        parts.update(g_parts[g])
        blob = np.empty((BLOB_SIZE,), bf)
        for nm, n in _BLOB_SPEC:
            off, _ = BLOB_LAYOUT[nm]
            blob[off:off + n] = np.ascontiguousarray(parts[nm]).reshape(-1)
        in_maps.append({"blob": blob})
    return in_maps


def kernel(x, encoded_data, freqs, attn_bias, Wq, Wk, Wv, Wo,
           q_scale, k_scale):
    nc = get_nc()
    in_maps = make_in_maps(x, encoded_data, freqs, attn_bias,
                           Wq, Wk, Wv, Wo, q_scale, k_scale)
    res = run_bass_kernel_spmd(nc, in_maps, core_ids=list(range(8)))
    out = np.empty((B, T, C), np.float32)
    for b in range(B):
        out[b] = (res.results[2 * b]["out"].astype(np.float32)
                  + res.results[2 * b + 1]["out"].astype(np.float32))
    return out
